# revision 9
# baseline (speedup 1.0000x reference)
"""APPNP-style 3-hop GNN message passing on 8 Trainium2 NeuronCores.

Strategy: shard destination rows across cores (each core owns N/8 nodes and
all edges pointing into them). Per hop: AllGather the node embeddings (bf16)
into a replicated DRAM table, dma_gather the source rows per edge, and
segment-sum into destination rows via one-hot matmuls on TensorE (S[e,j] =
val[e] * (local_row[e] == j), built in one DVE tensor_scalar per 128-edge
chunk). Edge slots are statically laid out per (row-tile x source-chunk)
group with capacity = max over cores, so the SPMD graph is identical on all
cores (no runtime registers; padding gathers dummy row 0 with val 0).
"""
import sys
import numpy as np

sys.path.insert(0, "/opt/trn_rl_repo")

P = 128
N_CORES = 8
DIM = 128
N_HOPS = 3
ST = 8          # row-tiles per supertile (PSUM bound)
CHUNK_ROWS = 30720  # gather table rows addressable per call (int16 idx)
SUB_CHUNKS = 7  # chunks per dma_gather sub-call (SWDGE ring is ~1024 descs)
N_QUEUES = 1    # SWDGE queues to round-robin gathers across

_f32 = None  # filled on first _bass() call
_CACHE = {}


def _bass_mods():
    from concourse import bacc, mybir, tile
    from concourse.bass_utils import run_bass_kernel_spmd
    from concourse.bass_interp import get_hw_module
    return bacc, mybir, tile, run_bass_kernel_spmd, get_hw_module


def _ceil128(x):
    return (x + 127) // 128 * 128


def host_prep(adj_rows, adj_cols, adj_vals, n_nodes_pad, per_core):
    """Bucket edges by (core, row-tile, col-chunk); build static slot layout.

    Returns (plan, per_core_arrays). plan is shared across cores; arrays are
    per-core [idx_wrapped int16 [128, TOT/16], lrv f32 [128, TOT/128] x2].
    """
    r_tiles = per_core // P
    n_chunks = (n_nodes_pad + CHUNK_ROWS - 1) // CHUNK_ROWS
    rows = adj_rows.astype(np.int64)
    cols = adj_cols.astype(np.int64)

    core = rows // per_core
    rt_g = rows >> 7                      # global row tile == core*r_tiles + rt
    lr = (rows & 127).astype(np.float32)
    ck = cols // CHUNK_ROWS
    key = rt_g * n_chunks + ck            # [0, 8*r_tiles*n_chunks)
    n_groups_g = N_CORES * r_tiles * n_chunks
    order = np.argsort(key, kind="stable")
    key_s = key[order]
    counts_g = np.bincount(key_s, minlength=n_groups_g)
    counts = counts_g.reshape(N_CORES, r_tiles, n_chunks)
    cap = _ceil128(counts.max(axis=0))            # [r_tiles, n_chunks]
    # every row-tile gets at least one chunk so PSUM is always written
    none = cap.sum(axis=1) == 0
    cap[none, 0] = P

    # slot layout: supertile-major, then col-chunk, then row-tile
    n_st = (r_tiles + ST - 1) // ST
    group_off = np.zeros((r_tiles, n_chunks), np.int64)
    calls = []  # (ck, slot_off, n_idx, [(chunk_rt, n_chunks_of_rt)...] per rt)
    pos = 0
    for st in range(n_st):
        rts = range(st * ST, min(st * ST + ST, r_tiles))
        for k in range(n_chunks):
            call_off = pos
            rt_spans = []
            for rt in rts:
                group_off[rt, k] = pos
                c = int(cap[rt, k])
                if c:
                    rt_spans.append((rt, c // P))
                pos += c
            if pos > call_off:
                calls.append((st, k, call_off, pos - call_off, rt_spans))
    tot = pos
    tot_chunks = tot // P

    # per-core slot arrays
    group_start_sorted = np.zeros(n_groups_g + 1, np.int64)
    np.cumsum(counts_g, out=group_start_sorted[1:])
    rank = np.arange(len(order)) - group_start_sorted[key_s]
    # target slot for each (sorted) edge
    rt_l = (rt_g % r_tiles)
    tgt = group_off[rt_l[order], ck[order]] + rank
    core_s = core[order]

    idx16 = np.zeros((N_CORES, tot), np.int16)
    lr_a = np.zeros((N_CORES, tot), np.float32)
    val_a = np.zeros((N_CORES, tot), np.float32)
    local_col = (cols - ck * CHUNK_ROWS).astype(np.int16)
    for c in range(N_CORES):
        m = core_s == c
        t = tgt[m]
        oc = order[m]
        idx16[c, t] = local_col[oc]
        lr_a[c, t] = lr[oc]
        val_a[c, t] = adj_vals[oc]

    # device layouts
    idx_w = np.zeros((N_CORES, P, tot // 16), np.int16)
    for (_st, _k, off, n, _sp) in calls:
        blk = idx16[:, off:off + n].reshape(N_CORES, n // 16, 16)
        idx_w[:, :16, off // 16:(off + n) // 16] = np.swapaxes(blk, 1, 2)
    idx_w[:, 16:, :] = np.tile(idx_w[:, :16, :], (1, 7, 1))
    lr_w = np.swapaxes(lr_a.reshape(N_CORES, tot_chunks, P), 1, 2).copy()
    val_w = np.swapaxes(val_a.reshape(N_CORES, tot_chunks, P), 1, 2).copy()

    plan = dict(r_tiles=r_tiles, n_chunks=n_chunks, n_st=n_st, tot=tot,
                tot_chunks=tot_chunks, calls=calls, cap=cap,
                n_nodes_pad=n_nodes_pad, per_core=per_core)
    return plan, idx_w, lr_w, val_w


def build_graph(plan):
    bacc, mybir, tile, _, _ = _bass_mods()
    f32 = mybir.dt.float32
    bf16 = mybir.dt.bfloat16
    i16 = mybir.dt.int16

    r_tiles = plan["r_tiles"]
    n_st = plan["n_st"]
    tot = plan["tot"]
    tot_chunks = plan["tot_chunks"]
    calls = plan["calls"]
    cap = plan["cap"]
    n_chunks = plan["n_chunks"]
    per_core = plan["per_core"]
    n_pad = plan["n_nodes_pad"]

    build_graph._q = 0
    nc = bacc.Bacc("TRN2", target_bir_lowering=False, debug=False,
                   enable_asserts=False, num_devices=N_CORES,
                   num_swdge_queues=N_QUEUES)
    x0 = nc.dram_tensor("x0", [per_core, DIM], f32, kind="ExternalInput")
    t_tr = nc.dram_tensor("t_tr", [P, r_tiles], f32, kind="ExternalInput")
    idx_d = nc.dram_tensor("idx", [P, tot // 16], i16, kind="ExternalInput")
    lr_d = nc.dram_tensor("lr", [P, tot_chunks], f32, kind="ExternalInput")
    val_d = nc.dram_tensor("val", [P, tot_chunks], f32, kind="ExternalInput")
    iota_d = nc.dram_tensor("iota", [P, P], bf16, kind="ExternalInput")
    out = nc.dram_tensor("out", [per_core, N_HOPS + 1, DIM], f32,
                         kind="ExternalOutput")

    tables = [nc.dram_tensor(f"table{i}", [n_pad, DIM], bf16,
                             addr_space="Shared") for i in range(2)]
    ag_in = nc.dram_tensor("ag_in", [per_core, DIM], bf16)
    rg = [list(range(N_CORES))]

    # per-call max chunk count for staging pool sizing
    max_call_chunks = max(n // P for (_s, _k, _o, n, _sp) in calls)

    with tile.TileContext(nc) as tc:
        with tc.tile_pool(name="res", bufs=1) as res, \
             tc.tile_pool(name="io", bufs=4) as io, \
             tc.tile_pool(name="gat", bufs=12) as gat, \
             tc.tile_pool(name="smat", bufs=8) as smat, \
             tc.tile_pool(name="psum", bufs=1, space="PSUM") as psp:

            # ---- phase 0: ego output + bf16 cast + AllGather into table0
            tt = res.tile([P, r_tiles], f32, tag="tt")
            nc.sync.dma_start(out=tt[:], in_=t_tr[:, :])
            for r in range(r_tiles):
                xi = io.tile([P, DIM], f32, tag="xin")
                nc.sync.dma_start(out=xi[:], in_=x0[r * P:(r + 1) * P, :])
                ego = io.tile([P, DIM], f32, tag="ego")
                nc.scalar.activation(out=ego[:], in_=xi[:],
                                     func=mybir.ActivationFunctionType.Copy,
                                     scale=tt[:, r:r + 1])
                nc.sync.dma_start(out=out[r * P:(r + 1) * P, 0, :], in_=ego[:])
                xb = io.tile([P, DIM], bf16, tag="xb")
                nc.vector.tensor_copy(out=xb[:], in_=xi[:])
                nc.sync.dma_start(out=ag_in[r * P:(r + 1) * P, :], in_=xb[:])
            nc.gpsimd.collective_compute(
                "AllGather", mybir.AluOpType.bypass, replica_groups=rg,
                ins=[ag_in.ap().opt()], outs=[tables[0].ap().opt()])

            # ---- residents
            lr_s = res.tile([P, tot_chunks], f32, tag="lr")
            nc.sync.dma_start(out=lr_s[:], in_=lr_d[:, :])
            val_s = res.tile([P, tot_chunks], f32, tag="val")
            nc.sync.dma_start(out=val_s[:], in_=val_d[:, :])
            io_s = res.tile([P, P], bf16, tag="iota")
            nc.sync.dma_start(out=io_s[:], in_=iota_d[:, :])
            dec = res.tile([P, r_tiles], f32, tag="dec")
            nc.vector.tensor_scalar(out=dec[:], in0=tt[:], scalar1=-1.0,
                                    scalar2=1.0, op0=mybir.AluOpType.mult,
                                    op1=mybir.AluOpType.add)
            pw = res.tile([P, r_tiles], f32, tag="pw")
            nc.vector.memset(pw[:], 1.0)
            sc = res.tile([P, r_tiles], f32, tag="sc")

            # ---- hops
            for h in range(1, N_HOPS + 1):
                tin = tables[(h - 1) % 2]
                tout = tables[h % 2]
                nc.vector.tensor_tensor(out=pw[:], in0=pw[:], in1=dec[:],
                                        op=mybir.AluOpType.mult)
                nc.vector.tensor_tensor(out=sc[:], in0=tt[:], in1=pw[:],
                                        op=mybir.AluOpType.mult)

                for st in range(n_st):
                    rts = list(range(st * ST, min(st * ST + ST, r_tiles)))
                    ys = {}
                    for j, rt in enumerate(rts):
                        ybank = psp.tile([P, P], f32, tag=f"y{j}",
                                         space="PSUM")
                        ys[rt] = ybank[:]
                    # first/last chunk bookkeeping per row tile
                    nch_of = {rt: int(cap[rt].sum()) // P for rt in rts}
                    seen = {rt: 0 for rt in rts}

                    for (st_c, k, off, n, rt_spans) in calls:
                        if st_c != st:
                            continue
                        nch = n // P
                        ix = io.tile([P, max_call_chunks * 8], i16, tag="ix")
                        nc.sync.dma_start(
                            out=ix[:, :n // 16],
                            in_=idx_d[:, off // 16:(off + n) // 16])
                        tbl = tin[k * CHUNK_ROWS:
                                  min((k + 1) * CHUNK_ROWS, n_pad), :]
                        # chunk index -> row tile, flattened from rt_spans
                        chunk_rt = []
                        for rt, nch_rt in rt_spans:
                            chunk_rt += [rt] * nch_rt
                        subs = []
                        for c0 in range(0, nch, SUB_CHUNKS):
                            subs.append((c0, min(SUB_CHUNKS, nch - c0)))
                        for (c0, snch) in subs:
                            sn = snch * P
                            g = gat.tile([P, SUB_CHUNKS, P], bf16, tag="g")
                            nc.gpsimd.dma_gather(
                                g[:, :snch, :], tbl,
                                ix[:, (c0 * P) // 16:(c0 * P + sn) // 16],
                                sn, sn, DIM,
                                queue_num=build_graph._q % N_QUEUES)
                            build_graph._q += 1
                            for ci in range(snch):
                                rt = chunk_rt[c0 + ci]
                                j = off // P + c0 + ci
                                s = smat.tile([P, P], bf16, tag="s")
                                nc.vector.tensor_scalar(
                                    out=s[:], in0=io_s[:],
                                    scalar1=lr_s[:, j:j + 1],
                                    scalar2=val_s[:, j:j + 1],
                                    op0=mybir.AluOpType.is_equal,
                                    op1=mybir.AluOpType.mult)
                                nc.tensor.matmul(
                                    out=ys[rt], lhsT=s[:], rhs=g[:, ci, :],
                                    start=(seen[rt] == 0),
                                    stop=(seen[rt] == nch_of[rt] - 1))
                                seen[rt] += 1
                    # evict supertile
                    for rt in rts:
                        if h < N_HOPS:
                            yb = io.tile([P, DIM], bf16, tag="yb")
                            nc.vector.tensor_copy(out=yb[:], in_=ys[rt])
                            nc.sync.dma_start(
                                out=ag_in[rt * P:(rt + 1) * P, :], in_=yb[:])
                        ysc = io.tile([P, DIM], f32, tag="ysc")
                        nc.scalar.activation(
                            out=ysc[:], in_=ys[rt],
                            func=mybir.ActivationFunctionType.Copy,
                            scale=sc[:, rt:rt + 1])
                        nc.sync.dma_start(
                            out=out[rt * P:(rt + 1) * P, h, :], in_=ysc[:])
                if h < N_HOPS:
                    nc.gpsimd.collective_compute(
                        "AllGather", mybir.AluOpType.bypass, replica_groups=rg,
                        ins=[ag_in.ap().opt()], outs=[tout.ap().opt()])
    return nc


def make_in_maps(user_embed, item_embed, user_t, item_t,
                 adj_vals, adj_rows, adj_cols, n_nodes_pad, per_core):
    import ml_dtypes
    plan, idx_w, lr_w, val_w = host_prep(adj_rows, adj_cols, adj_vals,
                                         n_nodes_pad, per_core)
    n_real = user_embed.shape[0] + item_embed.shape[0]
    r_tiles = per_core // P
    x_full = np.zeros((n_nodes_pad, DIM), np.float32)
    x_full[:user_embed.shape[0]] = user_embed
    x_full[user_embed.shape[0]:n_real] = item_embed
    t_full = np.zeros((n_nodes_pad,), np.float32)
    t_full[:user_embed.shape[0]] = user_t[:, 0]
    t_full[user_embed.shape[0]:n_real] = item_t[:, 0]
    iota = np.broadcast_to(np.arange(P, dtype=np.float32), (P, P)) \
        .astype(ml_dtypes.bfloat16).copy()

    in_maps = []
    for c in range(N_CORES):
        sl = slice(c * per_core, (c + 1) * per_core)
        t_tr = t_full[sl].reshape(r_tiles, P).T.copy()
        in_maps.append({
            "x0": np.ascontiguousarray(x_full[sl]),
            "t_tr": t_tr,
            "idx": np.ascontiguousarray(idx_w[c]),
            "lr": np.ascontiguousarray(lr_w[c]),
            "val": np.ascontiguousarray(val_w[c]),
            "iota": iota,
        })
    return plan, in_maps


def run_compiled(nc, in_maps, n_cores):
    _, _, _, run_bass_kernel_spmd, get_hw_module = _bass_mods()
    nc.m = get_hw_module(nc.m)
    res = run_bass_kernel_spmd(nc, in_maps, core_ids=list(range(n_cores)))
    return res


def kernel(user_embed, item_embed, user_t, item_t, adj_vals, adj_rows,
           adj_cols):
    n_users = user_embed.shape[0]
    n_items = item_embed.shape[0]
    n_real = n_users + n_items
    per_core = _ceil128((n_real + N_CORES - 1) // N_CORES)
    n_pad = per_core * N_CORES

    plan, in_maps = make_in_maps(user_embed, item_embed, user_t, item_t,
                                 adj_vals, adj_rows, adj_cols, n_pad, per_core)
    nc = build_graph(plan)
    nc.compile()
    res = run_compiled(nc, in_maps, N_CORES)
    full = np.concatenate([res.results[c]["out"] for c in range(N_CORES)], 0)
    return full[:n_users], full[n_users:n_real]


if __name__ == "__main__":
    pass
